# revision 24
# baseline (speedup 1.0000x reference)
"""Trainium2 Bass kernel for nn_HKRPQParallelBlock (RPQ-quantized parallel
transformer block: LN -> in_proj (dequant GEMM) -> [MLP | SDPA] -> out_proj
(dequant GEMM) -> fold + residual).

Sharding (8 cores, zero device-to-device communication):
  - Each core computes a 896-row slice of in_proj (512 MLP rows + q/k/v rows of
    2 heads), cluster-aligned so the RPQ structure stays uniform.
  - Each core runs SDPA for its 2 heads over all 4 batches.
  - out_proj is sharded over its contraction dim (the 640 activation features
    this core produced); every core emits a full partial (1024, 4096) output
    and the host sums the 8 partials (+ residual).
  - The OUT_OUT->DIM fold (o[:, :1024] + o[:, 1024:]) is folded into the
    dequantized weight before the GEMM, halving out_proj FLOPs.

v3 vs v2 (PE is sequencer-bound: ~120ns issue overhead per matmul, and PE
stalled behind the weight-blob DMA stream and per-tile LN/exp latency):
  - One-hot index matrices ship as fp8e4 (0/1 exact) - halves their DMA.
  - Partial outputs ship as bf16 - halves output DMA (host sums in f32).
  - w1 dequant tiles interleave with in_proj(b0) f-tiles, w2 tiles with
    in_proj(b1), so dequant DMA streams behind in_proj compute.
  - LN is split into a stats/normalize pass (DVE+Act, fused rsqrt(var+eps))
    and a late transpose pass, so PE transposes never chase the DVE chain.
  - out_proj is skewed one batch late and split by token-half, interleaved
    between the two attention halves: while Act runs softmax exps, PE runs
    out_proj of the previous batch.
  - V-transposes run two heads per instruction; the softmax ones-column is
    shared between the heads ([v0 | ones | v1] layout, h1 reads cols 64:129).
  - exp->AV pipeline deepened to 2 kv-tiles; the softmax denominator
    broadcast runs on the idle Pool engine (partition_broadcast) instead of
    a PE ones-matmul.
"""

import os
import numpy as np
import concourse.bass as bass
import concourse.bacc as bacc
import concourse.tile as tile
import concourse.mybir as mybir
from concourse.bass_utils import run_bass_kernel_spmd
from concourse.masks import make_identity
from contextlib import ExitStack

F32 = mybir.dt.float32
F32R = mybir.dt.float32r
BF16 = mybir.dt.bfloat16
FP8 = mybir.dt.float8e4
AF = mybir.ActivationFunctionType
ALU = mybir.AluOpType

NCORES = 8
DIM = 1024
HEADS = 16
HD = 64
MLP = 4 * DIM                 # 4096
IN_OUT = MLP + 3 * DIM        # 7168
OUT_IN = MLP + DIM            # 5120
OUT_OUT = 2 * DIM             # 2048
R = 2
K = 64
NCB = 16
SUB_IN = 64
SUB_OUT = 320
IN_CLUSTERS = 112
OUT_CLUSTERS = 16
B, N = 4, 1024
TOK = B * N                   # 4096
EPS = 1e-5
SCALE = HD ** -0.5            # 0.125

F_TILES = 7                   # per-core in_proj feature tiles of 128 rows
MLP_PER_CORE = MLP // NCORES  # 512
HEADS_PER_CORE = 2
KK = 5                        # out_proj contraction tiles of 128 per core
F_ORDER = [4, 5, 6, 0, 1, 2, 3]   # q,k,v tiles first

NPBF16 = mybir.dt.np(BF16)
NPFP8 = mybir.dt.np(FP8)

# packed codebook-blob section sizes (bf16 elements)
ICB_T = 128 * R * 2 * 8 * 128          # per f-tile: 524288
OCB_T = 128 * R * 2 * KK * 128         # per out-tile: 327680
OCB_OFF = F_TILES * ICB_T              # 3670016
CB_TOTAL = OCB_OFF + 8 * OCB_T         # 6291456
# packed one-hot blob section sizes (fp8 elements)
IOH_T = 128 * R * 2 * 8 * 64           # per f-tile: 262144
OOH_OFF = F_TILES * IOH_T              # 1835008
OH_TOTAL = OOH_OFF + 8 * OCB_T         # 4456448

_BUILD_CACHE = {}


def _row_base(core, t):
    """Global in_proj row of the first row of per-core feature tile t."""
    if t < 4:
        return MLP_PER_CORE * core + 128 * t
    return MLP + DIM * (t - 4) + 128 * core


def _chunk_cols(core):
    """The 10 global out_proj contraction columns (as 64-wide chunks) this
    core owns, in rhs order: 8 MLP chunks then 2 attention chunks."""
    return [MLP_PER_CORE * core + 64 * k for k in range(8)] + \
           [MLP + 128 * core + 64 * k for k in range(2)]


def _build_nc():
    if "nc" in _BUILD_CACHE:
        return _BUILD_CACHE["nc"]

    nc = bacc.Bacc("TRN2", target_bir_lowering=False, debug=False,
                   num_devices=NCORES)

    x_d = nc.dram_tensor("x4096", (TOK, DIM), BF16, kind="ExternalInput")
    cb_d = nc.dram_tensor("cbblob", (CB_TOTAL,), BF16, kind="ExternalInput")
    oh_d = nc.dram_tensor("ohblob", (OH_TOTAL,), FP8, kind="ExternalInput")
    o_d = nc.dram_tensor("o_t", (DIM, TOK), BF16, kind="ExternalOutput")

    with ExitStack() as ctx, nc.allow_low_precision(reason="bf16 matmul feeds"):
        tc = ctx.enter_context(tile.TileContext(nc))
        const = ctx.enter_context(tc.tile_pool(name="const", bufs=1))
        wpool = ctx.enter_context(tc.tile_pool(name="wpool", bufs=1))
        stage = ctx.enter_context(tc.tile_pool(name="stage", bufs=2))
        lnp = ctx.enter_context(tc.tile_pool(name="lnp", bufs=2))
        work = ctx.enter_context(tc.tile_pool(name="work", bufs=4))
        small = ctx.enter_context(tc.tile_pool(name="small", bufs=2))
        psA = ctx.enter_context(tc.tile_pool(name="psA", bufs=4, space="PSUM"))
        psT = ctx.enter_context(tc.tile_pool(name="psT", bufs=2, space="PSUM"))
        psV = ctx.enter_context(tc.tile_pool(name="psV", bufs=1, space="PSUM"))
        ptpool = ctx.enter_context(tc.tile_pool(name="ptpool", bufs=4))

        ident_f = const.tile([128, 128], F32, tag="ident_f")
        make_identity(nc, ident_f[:])
        ident_b = const.tile([128, 128], BF16, tag="ident_b")
        nc.vector.tensor_copy(ident_b[:], ident_f[:])
        eps_c = const.tile([128, 1], F32, tag="eps_c")
        nc.gpsimd.memset(eps_c[:], EPS)

        winT = wpool.tile([128, 8, F_TILES * 128], BF16, name="winT",
                          tag="winT")
        wfoldT = wpool.tile([128, KK, 1024], BF16, name="wfT", tag="wfT")

        # ---------------- emission helpers ------------------------------
        def emit_ln_passA(b, xns):
            """LayerNorm stats + normalize for batch b into 8 xn tiles."""
            for tt in range(8):
                xt = stage.tile([128, DIM], BF16, name="xt", tag="xt")
                nc.sync.dma_start(
                    xt[:], x_d.ap()[b * N + tt * 128: b * N + (tt + 1) * 128, :])
                bstat = small.tile([128, 2, 6], F32, name="bstat", tag="bstat")
                nc.vector.bn_stats(bstat[:, 0, :], xt[:, :512])
                nc.vector.bn_stats(bstat[:, 1, :], xt[:, 512:])
                baggr = small.tile([128, 2], F32, name="baggr", tag="baggr")
                nc.vector.bn_aggr(baggr[:], bstat[:])
                sd = small.tile([128, 1], F32, name="sd", tag="sd")
                nc.scalar.activation(sd[:], baggr[:, 1:2], AF.Sqrt,
                                     bias=eps_c[:])
                rs = small.tile([128, 1], F32, name="rs", tag="rs")
                nc.vector.reciprocal(rs[:], sd[:])
                xn = lnp.tile([128, DIM], BF16, name=f"xn{tt}", tag=f"xn{tt}")
                nc.vector.tensor_scalar(xn[:], xt[:], baggr[:, 0:1], rs[:],
                                        op0=ALU.subtract, op1=ALU.mult)
                xns[tt] = xn

        def emit_ln_passB_tt(xns, xnT, tt):
            """PE-transpose one normalized tile into xnT layout.

            4 transpose outputs share one PSUM bank at different offsets, so
            each drain is a single [128, 4, 128] strided copy."""
            for g in range(2):
                pst = psT.tile([128, 512], BF16, name="pstb", tag="pstr")
                for k in range(4):
                    d = 4 * g + k
                    nc.tensor.matmul(pst[:, k * 128:(k + 1) * 128],
                                     xns[tt][:, d * 128:(d + 1) * 128],
                                     ident_b[:], is_transpose=True,
                                     start=True, stop=True)
                dst = xnT[:, 4 * g:4 * g + 4, tt * 128:(tt + 1) * 128]
                src = pst[:].rearrange("p (k f) -> p k f", k=4)
                if (tt + g) % 2:
                    nc.scalar.copy(dst, src)
                else:
                    nc.vector.tensor_copy(dst, src)

        def emit_ln_passB(xns, xnT):
            for tt in range(8):
                emit_ln_passB_tt(xns, xnT, tt)

        def emit_w1_tile(t):
            """in_proj dequant f-tile t: one-hot x codebook matmuls."""
            cbt = stage.tile([128, R, 2, 8, 128], BF16, name="icbt",
                             tag="icbt")
            nc.gpsimd.dma_start(
                cbt[:].rearrange("p r h d f -> p (r h d f)"),
                cb_d.ap()[t * ICB_T:(t + 1) * ICB_T]
                .rearrange("(p f) -> p f", p=128))
            oht = stage.tile([128, R, 2, 8, 64], FP8, name="ioht",
                             tag="ioht")
            nc.gpsimd.dma_start(
                oht[:].rearrange("p r h d f -> p (r h d f)"),
                oh_d.ap()[t * IOH_T:(t + 1) * IOH_T]
                .rearrange("(p f) -> p f", p=128))
            for g in range(2):
                ps = psT.tile([128, 512], F32, name="pstr", tag="pstr")
                for k in range(4):
                    d = 4 * g + k
                    for h in range(2):
                        for r in range(R):
                            nc.tensor.matmul(
                                ps[:, k * 128 + h * 64:k * 128 + h * 64 + 64],
                                cbt[:, r, h, d, :], oht[:, r, h, d, :],
                                start=(r == 0), stop=(r == 1))
                dst = winT[:, 4 * g:4 * g + 4, t * 128:(t + 1) * 128]
                src = ps[:].rearrange("p (k f) -> p k f", k=4)
                if g:
                    nc.scalar.copy(dst, src)
                else:
                    nc.vector.tensor_copy(dst, src)

        def emit_w2_tile(ot):
            """out_proj dequant out-tile ot: levels+fold accumulated."""
            cbt = stage.tile([128, R, 2, KK, 128], BF16, name="ocbt",
                             tag="ocbt")
            nc.gpsimd.dma_start(
                cbt[:].rearrange("p r f k c -> p (r f k c)"),
                cb_d.ap()[OCB_OFF + ot * OCB_T:OCB_OFF + (ot + 1) * OCB_T]
                .rearrange("(p f) -> p f", p=128))
            oht = stage.tile([128, R, 2, KK, 128], FP8, name="ooht",
                             tag="ooht")
            nc.gpsimd.dma_start(
                oht[:].rearrange("p r f k c -> p (r f k c)"),
                oh_d.ap()[OOH_OFF + ot * OCB_T:OOH_OFF + (ot + 1) * OCB_T]
                .rearrange("(p f) -> p f", p=128))
            for g, kks in ((0, range(4)), (1, range(4, KK))):
                width = 128 * len(kks)
                ps = psT.tile([128, 512], F32, name="pstr", tag="pstr")
                for j, kk in enumerate(kks):
                    i = 0
                    for r in range(R):
                        for fh in range(2):
                            nc.tensor.matmul(
                                ps[:, j * 128:(j + 1) * 128],
                                cbt[:, r, fh, kk, :], oht[:, r, fh, kk, :],
                                start=(i == 0), stop=(i == 3))
                            i += 1
                k0 = kks[0]
                dst = wfoldT[:, k0:k0 + len(kks), ot * 128:(ot + 1) * 128]
                src = ps[:, 0:width].rearrange("p (k f) -> p k f",
                                               k=len(kks))
                if g:
                    nc.scalar.copy(dst, src)
                else:
                    nc.vector.tensor_copy(dst, src)

        def emit_inproj_tile(xnT, hT, f):
            pss = [psA.tile([128, 512], F32, name="mm", tag="mm")
                   for _ in range(2)]
            for d in range(8):
                for qc in range(2):
                    nc.tensor.matmul(
                        pss[qc][:], winT[:, d, f * 128:(f + 1) * 128],
                        xnT[:, d, qc * 512:(qc + 1) * 512],
                        start=(d == 0), stop=(d == 7))
            nc.scalar.copy(hT[f][:, 0:512], pss[0][:])
            nc.vector.tensor_copy(hT[f][:, 512:1024], pss[1][:])

        def emit_vtransp(hT, par):
            # V^T for both heads per instruction; shared ones col at 64:
            # vaug free layout = [v0 (0:64) | ones (64) | v1 (65:129)]
            vaug = wpool.tile([128, 8, 130], BF16, name="vaug",
                              tag=f"vaug{par}")
            for g in range(2):
                psv = psT.tile([128, 512], BF16, name="pstb", tag="pstr")
                for k in range(4):
                    kvt = 4 * g + k
                    nc.tensor.matmul(psv[:, k * 128:(k + 1) * 128],
                                     hT[6][:, kvt * 128:(kvt + 1) * 128],
                                     ident_b[:], is_transpose=True,
                                     start=True, stop=True)
                src = psv[:].rearrange("p (k f) -> p k f", k=4)
                nc.vector.tensor_copy(vaug[:, 4 * g:4 * g + 4, 0:64],
                                      src[:, :, 0:64])
                nc.scalar.copy(vaug[:, 4 * g:4 * g + 4, 65:129],
                               src[:, :, 64:128])
            nc.vector.memset(vaug[:, :, 64:65], 1.0)
            nc.vector.memset(vaug[:, :, 129:130], 1.0)
            return vaug

        def emit_attn_qc(hT, vaug, xaT, qc):
            psavs = [psV.tile([65, 512], F32, name=f"av{hh}",
                              tag=f"av{hh}") for hh in range(2)]
            pipe = []          # [(kvt, [ptk_h0, ptk_h1])]
            for kvt in range(8):
                cur = []
                for hh in range(HEADS_PER_CORE):
                    lo, hi = hh * 64, (hh + 1) * 64
                    ps = psA.tile([128, 512], F32, name="mm", tag="mm")
                    nc.tensor.matmul(
                        ps[:], hT[5][lo:hi, kvt * 128:(kvt + 1) * 128],
                        hT[4][lo:hi, qc * 512:(qc + 1) * 512],
                        start=True, stop=True)
                    ptk = ptpool.tile([128, 512], BF16, name=f"PT{hh}",
                                      tag=f"PT{hh}")
                    nc.scalar.activation(ptk[:], ps[:], AF.Exp,
                                         scale=SCALE)
                    cur.append(ptk)
                pipe.append((kvt, cur))
                if len(pipe) > 3:
                    okvt, tiles = pipe.pop(0)
                    for hh in range(HEADS_PER_CORE):
                        nc.tensor.matmul(
                            psavs[hh][:],
                            vaug[:, okvt, hh * 65:hh * 65 + 65],
                            tiles[hh][:],
                            start=(okvt == 0), stop=False)
            for okvt, tiles in pipe:
                for hh in range(HEADS_PER_CORE):
                    nc.tensor.matmul(
                        psavs[hh][:],
                        vaug[:, okvt, hh * 65:hh * 65 + 65],
                        tiles[hh][:],
                        start=(okvt == 0), stop=(okvt == 7))
            for hh in range(HEADS_PER_CORE):
                lo, hi = hh * 64, (hh + 1) * 64
                psav = psavs[hh]
                # rows 0:64 = AV, row 64 = softmax denominator
                rec = small.tile([1, 512], F32, name="rec", tag="rec")
                nc.vector.reciprocal(rec[:], psav[64:65, :])
                bc = small.tile([64, 512], F32, name="bc_sb", tag="bc_sb")
                nc.gpsimd.partition_broadcast(bc[:], rec[:])
                nc.vector.tensor_tensor(
                    xaT[lo:hi, qc * 512:(qc + 1) * 512],
                    psav[0:64, :], bc[:], op=ALU.mult)

        def emit_outproj_qc(b, hT, xaT, qc):
            for ot in range(8):
                ps = psA.tile([128, 512], F32, name="mm", tag="mm")
                for kk in range(KK):
                    rhs = hT[kk] if kk < 4 else xaT
                    nc.tensor.matmul(
                        ps[:], wfoldT[:, kk, ot * 128:(ot + 1) * 128],
                        rhs[:, qc * 512:(qc + 1) * 512],
                        start=(kk == 0), stop=(kk == KK - 1))
                osb = work.tile([128, 512], BF16, name="osb", tag="osbh")
                if ot % 2:
                    nc.scalar.copy(osb[:], ps[:])
                else:
                    nc.vector.tensor_copy(osb[:], ps[:])
                nc.sync.dma_start(
                    o_d.ap()[ot * 128:(ot + 1) * 128,
                             b * N + qc * 512: b * N + (qc + 1) * 512],
                    osb[:])

        # ---------------- emission order --------------------------------
        def alloc_xnT(par):
            return wpool.tile([128, 8, N], BF16, name="xnT",
                              tag=f"xnTp{par}")

        def alloc_hT(par):
            return [wpool.tile([128, N], BF16, name=f"hT{f}",
                               tag=f"hT{f}p{par}") for f in range(F_TILES)]

        def alloc_xaT(par):
            return wpool.tile([128, N], BF16, name="xaT", tag=f"xaTp{par}")

        # BASS_REPEAT>1 re-emits the body k times for wall-delta timing
        for _rep in range(int(os.environ.get("BASS_REPEAT", "1"))):
            xnT = [alloc_xnT(0), alloc_xnT(1)]
            hT = [alloc_hT(0), alloc_hT(1)]
            xaT = [alloc_xaT(0), alloc_xaT(1)]
            xns = [None] * 8

            # bootstrap: w1 dequant tiles fill PE while the LN chain streams
            emit_w1_tile(4)
            emit_ln_passA(0, xns)
            emit_w1_tile(5)
            w1_rest = [6, 0, 1, 2, 3]
            for tt in range(8):
                emit_ln_passB_tt(xns, xnT[0], tt)
                if tt % 2 == 1 and w1_rest:
                    emit_w1_tile(w1_rest.pop(0))
            while w1_rest:
                emit_w1_tile(w1_rest.pop(0))
            for b in range(B):
                par = b % 2
                # in_proj, interleaved with the w2 dequant stream on b1
                for i, f in enumerate(F_ORDER):
                    if b == 1:
                        emit_w2_tile(i)
                    emit_inproj_tile(xnT[par], hT[par], f)
                if b == 1:
                    emit_w2_tile(7)
                vaug = emit_vtransp(hT[par], par)
                emit_attn_qc(hT[par], vaug, xaT[par], 0)
                if b > 0:
                    emit_outproj_qc(b - 1, hT[1 - par], xaT[1 - par], 0)
                if b + 1 < B:
                    emit_ln_passA(b + 1, xns)
                emit_attn_qc(hT[par], vaug, xaT[par], 1)
                if b > 0:
                    emit_outproj_qc(b - 1, hT[1 - par], xaT[1 - par], 1)
                if b + 1 < B:
                    emit_ln_passB(xns, xnT[1 - par])
            emit_outproj_qc(3, hT[1], xaT[1], 0)
            emit_outproj_qc(3, hT[1], xaT[1], 1)

    nc.compile()
    _BUILD_CACHE["nc"] = nc
    return nc


def make_in_maps(x, in_codebooks, in_indices, out_codebooks, out_indices):
    """Host-side input marshalling: per-core one-hot index matrices (fp8) and
    block-diagonal codebook tiles (bf16), plus the flattened activations.

    Pure layout/encoding transforms - all arithmetic (dequant sums, GEMMs,
    LN, SDPA) runs on device.
    """
    x4096 = np.ascontiguousarray(np.asarray(x).reshape(TOK, DIM)
                                 .astype(NPBF16))
    in_cb = np.asarray(in_codebooks, np.float32)
    in_idx = np.asarray(in_indices)
    out_cb = np.asarray(out_codebooks, np.float32)
    out_idx = np.asarray(out_indices)
    eye = np.arange(K)

    in_maps = []
    for c in range(NCORES):
        # ---- in_proj: icb (7,128,R,2,8,128), ioh (7,128,R,2,8,64) ----
        rows = np.stack([np.arange(_row_base(c, t), _row_base(c, t) + 128)
                         for t in range(F_TILES)])            # (7,128)
        cl0 = np.array([_row_base(c, t) // 64 for t in range(F_TILES)])

        ivc = in_idx[:, rows, :]                              # (R,7,128,16)
        oh = (ivc[..., None] == eye).astype(NPFP8)            # (R,7,128,16,64)
        # axes: (r, t, (h,m), (d,ci), k) -> (t, ci, k, r, h, d, m)
        oh = oh.reshape(R, F_TILES, 2, 64, 8, 2, K)
        ioh = np.ascontiguousarray(
            oh.transpose(1, 5, 6, 0, 2, 4, 3)                 # t,ci,k,r,h,d,m
            .reshape(F_TILES, 128, R, 2, 8, 64))

        # cb tiles: (r, t, h, d, ci, k, s) from clusters cl0[t]+h
        cl_ids = cl0[:, None] + np.array([0, 1])              # (7,2)
        cbs = in_cb[:, cl_ids]                                # (R,7,2,16,64,64)
        cbs = cbs.reshape(R, F_TILES, 2, 8, 2, K, SUB_IN)
        icb = np.zeros((F_TILES, 128, R, 2, 8, 128), np.float32)
        for ci in range(2):
            # (r,t,h,d,k,s) -> (t,k,r,h,d,s)
            blk = cbs[:, :, :, :, ci].transpose(1, 4, 0, 2, 3, 5)
            icb[:, ci * 64:(ci + 1) * 64, :, :, :,
                ci * 64:(ci + 1) * 64] = blk
        icb = icb.astype(NPBF16)

        # ---- out_proj: ocb (bf16) / ooh (fp8), (8,128,R,2,5,128) ---------
        cols = _chunk_cols(c)
        gcbk = np.array([g // SUB_OUT for g in cols])          # (10,)
        gsub = np.array([(g % SUB_OUT) // 64 for g in cols])   # (10,)

        # per-chunk codebook slices: (R, 16 clusters, 10, K, 64)
        sel = np.empty((R, OUT_CLUSTERS, 10, K, 64), np.float32)
        for i in range(10):
            sel[:, :, i] = out_cb[:, :, gcbk[i], :,
                                  64 * gsub[i]: 64 * gsub[i] + 64]
        # (r, (fh,ot) cluster, (kk,ci), k, s) -> (ot, ci, k, r, fh, kk, s)
        sel = sel.reshape(R, 2, 8, KK, 2, K, 64)
        ocb = np.zeros((8, 128, R, 2, KK, 128), np.float32)
        for ci in range(2):
            # (r,fh,ot,kk,k,s) -> (ot,k,r,fh,kk,s)
            blk = sel[:, :, :, :, ci].transpose(2, 4, 0, 1, 3, 5)
            ocb[:, ci * 64:(ci + 1) * 64, :, :, :,
                ci * 64:(ci + 1) * 64] = blk
        ocb = ocb.astype(NPBF16)

        ov = out_idx[:, :, gcbk]                               # (R,2048,10)
        ooh_raw = (ov[..., None] == eye).astype(NPFP8)         # (R,2048,10,64)
        # rows: (r, (fh,ot,m), (kk,ci), k) -> (ot, ci, k, r, fh, kk, m)
        ooh_raw = ooh_raw.reshape(R, 2, 8, 128, KK, 2, K)
        ooh = np.ascontiguousarray(
            ooh_raw.transpose(2, 5, 6, 0, 1, 4, 3)
            .reshape(8, 128, R, 2, KK, 128))

        cbblob = np.concatenate([
            np.ascontiguousarray(icb).ravel(),
            np.ascontiguousarray(ocb).ravel()])
        ohblob = np.concatenate([ioh.ravel(), ooh.ravel()])
        assert cbblob.shape[0] == CB_TOTAL and cbblob.dtype == NPBF16
        assert ohblob.shape[0] == OH_TOTAL and ohblob.dtype == NPFP8
        in_maps.append({"x4096": x4096, "cbblob": cbblob, "ohblob": ohblob})
    return in_maps


def combine_outputs(x, results):
    o_sum = np.zeros((DIM, TOK), np.float32)
    for rmap in results:
        o_sum += np.asarray(rmap["o_t"]).astype(np.float32)
    out = np.asarray(x).reshape(TOK, DIM).astype(np.float32) + o_sum.T
    return out.reshape(B, N, DIM).astype(np.float32)


def kernel(x, in_codebooks, in_indices, out_codebooks, out_indices):
    nc = _build_nc()
    in_maps = make_in_maps(x, in_codebooks, in_indices,
                           out_codebooks, out_indices)
    res = run_bass_kernel_spmd(nc, in_maps, core_ids=list(range(NCORES)))
    return combine_outputs(x, [res.results[c] for c in range(NCORES)])


# revision 26
# speedup vs baseline: 1.0038x; 1.0038x over previous
"""Trainium2 Bass kernel for nn_HKRPQParallelBlock (RPQ-quantized parallel
transformer block: LN -> in_proj (dequant GEMM) -> [MLP | SDPA] -> out_proj
(dequant GEMM) -> fold + residual).

Sharding (8 cores, zero device-to-device communication):
  - Each core computes a 896-row slice of in_proj (512 MLP rows + q/k/v rows of
    2 heads), cluster-aligned so the RPQ structure stays uniform.
  - Each core runs SDPA for its 2 heads over all 4 batches.
  - out_proj is sharded over its contraction dim (the 640 activation features
    this core produced); every core emits a full partial (1024, 4096) output
    and the host sums the 8 partials (+ residual).
  - The OUT_OUT->DIM fold (o[:, :1024] + o[:, 1024:]) is folded into the
    dequantized weight before the GEMM, halving out_proj FLOPs.

v3 vs v2 (PE is sequencer-bound: ~120ns issue overhead per matmul, and PE
stalled behind the weight-blob DMA stream and per-tile LN/exp latency):
  - One-hot index matrices ship as fp8e4 (0/1 exact) - halves their DMA.
  - Partial outputs ship as bf16 - halves output DMA (host sums in f32).
  - w1 dequant tiles interleave with in_proj(b0) f-tiles, w2 tiles with
    in_proj(b1), so dequant DMA streams behind in_proj compute.
  - LN is split into a stats/normalize pass (DVE+Act, fused rsqrt(var+eps))
    and a late transpose pass, so PE transposes never chase the DVE chain.
  - out_proj is skewed one batch late and split by token-half, interleaved
    between the two attention halves: while Act runs softmax exps, PE runs
    out_proj of the previous batch.
  - V-transposes run two heads per instruction; the softmax ones-column is
    shared between the heads ([v0 | ones | v1] layout, h1 reads cols 64:129).
  - exp->AV pipeline deepened to 2 kv-tiles; the softmax denominator
    broadcast runs on the idle Pool engine (partition_broadcast) instead of
    a PE ones-matmul.
"""

import os
import numpy as np
import concourse.bass as bass
import concourse.bacc as bacc
import concourse.tile as tile
import concourse.mybir as mybir
from concourse.bass_utils import run_bass_kernel_spmd
from concourse.masks import make_identity
from contextlib import ExitStack

F32 = mybir.dt.float32
F32R = mybir.dt.float32r
BF16 = mybir.dt.bfloat16
FP8 = mybir.dt.float8e4
AF = mybir.ActivationFunctionType
ALU = mybir.AluOpType

NCORES = 8
DIM = 1024
HEADS = 16
HD = 64
MLP = 4 * DIM                 # 4096
IN_OUT = MLP + 3 * DIM        # 7168
OUT_IN = MLP + DIM            # 5120
OUT_OUT = 2 * DIM             # 2048
R = 2
K = 64
NCB = 16
SUB_IN = 64
SUB_OUT = 320
IN_CLUSTERS = 112
OUT_CLUSTERS = 16
B, N = 4, 1024
TOK = B * N                   # 4096
EPS = 1e-5
SCALE = HD ** -0.5            # 0.125

F_TILES = 7                   # per-core in_proj feature tiles of 128 rows
MLP_PER_CORE = MLP // NCORES  # 512
HEADS_PER_CORE = 2
KK = 5                        # out_proj contraction tiles of 128 per core
F_ORDER = [4, 5, 6, 0, 1, 2, 3]   # q,k,v tiles first

NPBF16 = mybir.dt.np(BF16)
NPFP8 = mybir.dt.np(FP8)

# packed codebook-blob section sizes (bf16 elements)
ICB_T = 128 * R * 2 * 8 * 128          # per f-tile: 524288
OCB_T = 128 * R * 2 * KK * 128         # per out-tile: 327680
OCB_OFF = F_TILES * ICB_T              # 3670016
CB_TOTAL = OCB_OFF + 8 * OCB_T         # 6291456
# packed one-hot blob section sizes (fp8 elements)
IOH_T = 128 * R * 2 * 8 * 64           # per f-tile: 262144
OOH_OFF = F_TILES * IOH_T              # 1835008
OH_TOTAL = OOH_OFF + 8 * OCB_T         # 4456448

_BUILD_CACHE = {}


def _row_base(core, t):
    """Global in_proj row of the first row of per-core feature tile t."""
    if t < 4:
        return MLP_PER_CORE * core + 128 * t
    return MLP + DIM * (t - 4) + 128 * core


def _chunk_cols(core):
    """The 10 global out_proj contraction columns (as 64-wide chunks) this
    core owns, in rhs order: 8 MLP chunks then 2 attention chunks."""
    return [MLP_PER_CORE * core + 64 * k for k in range(8)] + \
           [MLP + 128 * core + 64 * k for k in range(2)]


def _build_nc():
    if "nc" in _BUILD_CACHE:
        return _BUILD_CACHE["nc"]

    nc = bacc.Bacc("TRN2", target_bir_lowering=False, debug=False,
                   num_devices=NCORES)

    x_d = nc.dram_tensor("x4096", (TOK, DIM), BF16, kind="ExternalInput")
    cb_d = nc.dram_tensor("cbblob", (CB_TOTAL,), BF16, kind="ExternalInput")
    oh_d = nc.dram_tensor("ohblob", (OH_TOTAL,), FP8, kind="ExternalInput")
    o_d = nc.dram_tensor("o_t", (DIM, TOK), BF16, kind="ExternalOutput")

    with ExitStack() as ctx, nc.allow_low_precision(reason="bf16 matmul feeds"):
        tc = ctx.enter_context(tile.TileContext(nc))
        const = ctx.enter_context(tc.tile_pool(name="const", bufs=1))
        wpool = ctx.enter_context(tc.tile_pool(name="wpool", bufs=1))
        stage = ctx.enter_context(tc.tile_pool(name="stage", bufs=2))
        lnp = ctx.enter_context(tc.tile_pool(name="lnp", bufs=2))
        work = ctx.enter_context(tc.tile_pool(name="work", bufs=4))
        small = ctx.enter_context(tc.tile_pool(name="small", bufs=2))
        psA = ctx.enter_context(tc.tile_pool(name="psA", bufs=4, space="PSUM"))
        psT = ctx.enter_context(tc.tile_pool(name="psT", bufs=2, space="PSUM"))
        psV = ctx.enter_context(tc.tile_pool(name="psV", bufs=1, space="PSUM"))
        ptpool = ctx.enter_context(tc.tile_pool(name="ptpool", bufs=4))

        ident_f = const.tile([128, 128], F32, tag="ident_f")
        make_identity(nc, ident_f[:])
        ident_b = const.tile([128, 128], BF16, tag="ident_b")
        nc.vector.tensor_copy(ident_b[:], ident_f[:])
        eps_c = const.tile([128, 1], F32, tag="eps_c")
        nc.gpsimd.memset(eps_c[:], EPS)
        ones_f = const.tile([128, 1], F32, tag="ones_f")
        nc.gpsimd.memset(ones_f[:], 1.0)
        ones_r = const.tile([1, 64], F32R, tag="ones_r")
        nc.vector.tensor_copy(ones_r[:], ones_f[:1, :].to_broadcast([1, 64]))

        winT = wpool.tile([128, 8, F_TILES * 128], BF16, name="winT",
                          tag="winT")
        wfoldT = wpool.tile([128, KK, 1024], BF16, name="wfT", tag="wfT")

        # ---------------- emission helpers ------------------------------
        def emit_ln_passA(b, xns):
            """LayerNorm stats + normalize for batch b into 8 xn tiles."""
            for tt in range(8):
                xt = stage.tile([128, DIM], BF16, name="xt", tag="xt")
                nc.sync.dma_start(
                    xt[:], x_d.ap()[b * N + tt * 128: b * N + (tt + 1) * 128, :])
                bstat = small.tile([128, 2, 6], F32, name="bstat", tag="bstat")
                nc.vector.bn_stats(bstat[:, 0, :], xt[:, :512])
                nc.vector.bn_stats(bstat[:, 1, :], xt[:, 512:])
                baggr = small.tile([128, 2], F32, name="baggr", tag="baggr")
                nc.vector.bn_aggr(baggr[:], bstat[:])
                sd = small.tile([128, 1], F32, name="sd", tag="sd")
                nc.scalar.activation(sd[:], baggr[:, 1:2], AF.Sqrt,
                                     bias=eps_c[:])
                rs = small.tile([128, 1], F32, name="rs", tag="rs")
                nc.vector.reciprocal(rs[:], sd[:])
                xn = lnp.tile([128, DIM], BF16, name=f"xn{tt}", tag=f"xn{tt}")
                nc.vector.tensor_scalar(xn[:], xt[:], baggr[:, 0:1], rs[:],
                                        op0=ALU.subtract, op1=ALU.mult)
                xns[tt] = xn

        def emit_ln_passB_tt(xns, xnT, tt):
            """PE-transpose one normalized tile into xnT layout.

            4 transpose outputs share one PSUM bank at different offsets, so
            each drain is a single [128, 4, 128] strided copy."""
            for g in range(2):
                pst = psT.tile([128, 512], BF16, name="pstb", tag="pstr")
                for k in range(4):
                    d = 4 * g + k
                    nc.tensor.matmul(pst[:, k * 128:(k + 1) * 128],
                                     xns[tt][:, d * 128:(d + 1) * 128],
                                     ident_b[:], is_transpose=True,
                                     start=True, stop=True)
                dst = xnT[:, 4 * g:4 * g + 4, tt * 128:(tt + 1) * 128]
                src = pst[:].rearrange("p (k f) -> p k f", k=4)
                if (tt + g) % 2:
                    nc.scalar.copy(dst, src)
                else:
                    nc.vector.tensor_copy(dst, src)

        def emit_ln_passB(xns, xnT):
            for tt in range(8):
                emit_ln_passB_tt(xns, xnT, tt)

        def emit_w1_tile(t):
            """in_proj dequant f-tile t: one-hot x codebook matmuls."""
            cbt = stage.tile([128, R, 2, 8, 128], BF16, name="icbt",
                             tag="icbt")
            nc.gpsimd.dma_start(
                cbt[:].rearrange("p r h d f -> p (r h d f)"),
                cb_d.ap()[t * ICB_T:(t + 1) * ICB_T]
                .rearrange("(p f) -> p f", p=128))
            oht = stage.tile([128, R, 2, 8, 64], FP8, name="ioht",
                             tag="ioht")
            nc.gpsimd.dma_start(
                oht[:].rearrange("p r h d f -> p (r h d f)"),
                oh_d.ap()[t * IOH_T:(t + 1) * IOH_T]
                .rearrange("(p f) -> p f", p=128))
            for g in range(2):
                ps = psT.tile([128, 512], F32, name="pstr", tag="pstr")
                for k in range(4):
                    d = 4 * g + k
                    for h in range(2):
                        for r in range(R):
                            nc.tensor.matmul(
                                ps[:, k * 128 + h * 64:k * 128 + h * 64 + 64],
                                cbt[:, r, h, d, :], oht[:, r, h, d, :],
                                start=(r == 0), stop=(r == 1))
                dst = winT[:, 4 * g:4 * g + 4, t * 128:(t + 1) * 128]
                src = ps[:].rearrange("p (k f) -> p k f", k=4)
                if g:
                    nc.scalar.copy(dst, src)
                else:
                    nc.vector.tensor_copy(dst, src)

        def emit_w2_tile(ot):
            """out_proj dequant out-tile ot: levels+fold accumulated."""
            cbt = stage.tile([128, R, 2, KK, 128], BF16, name="ocbt",
                             tag="ocbt")
            nc.gpsimd.dma_start(
                cbt[:].rearrange("p r f k c -> p (r f k c)"),
                cb_d.ap()[OCB_OFF + ot * OCB_T:OCB_OFF + (ot + 1) * OCB_T]
                .rearrange("(p f) -> p f", p=128))
            oht = stage.tile([128, R, 2, KK, 128], FP8, name="ooht",
                             tag="ooht")
            nc.gpsimd.dma_start(
                oht[:].rearrange("p r f k c -> p (r f k c)"),
                oh_d.ap()[OOH_OFF + ot * OCB_T:OOH_OFF + (ot + 1) * OCB_T]
                .rearrange("(p f) -> p f", p=128))
            for g, kks in ((0, range(4)), (1, range(4, KK))):
                width = 128 * len(kks)
                ps = psT.tile([128, 512], F32, name="pstr", tag="pstr")
                for j, kk in enumerate(kks):
                    i = 0
                    for r in range(R):
                        for fh in range(2):
                            nc.tensor.matmul(
                                ps[:, j * 128:(j + 1) * 128],
                                cbt[:, r, fh, kk, :], oht[:, r, fh, kk, :],
                                start=(i == 0), stop=(i == 3))
                            i += 1
                k0 = kks[0]
                dst = wfoldT[:, k0:k0 + len(kks), ot * 128:(ot + 1) * 128]
                src = ps[:, 0:width].rearrange("p (k f) -> p k f",
                                               k=len(kks))
                if g:
                    nc.scalar.copy(dst, src)
                else:
                    nc.vector.tensor_copy(dst, src)

        def emit_inproj_tile(xnT, hT, f):
            pss = [psA.tile([128, 512], F32, name="mm", tag="mm")
                   for _ in range(2)]
            for d in range(8):
                for qc in range(2):
                    nc.tensor.matmul(
                        pss[qc][:], winT[:, d, f * 128:(f + 1) * 128],
                        xnT[:, d, qc * 512:(qc + 1) * 512],
                        start=(d == 0), stop=(d == 7))
            nc.scalar.copy(hT[f][:, 0:512], pss[0][:])
            nc.vector.tensor_copy(hT[f][:, 512:1024], pss[1][:])

        def emit_vtransp(hT, par):
            # V^T for both heads per instruction; shared ones col at 64:
            # vaug free layout = [v0 (0:64) | ones (64) | v1 (65:129)]
            vaug = wpool.tile([128, 8, 130], BF16, name="vaug",
                              tag=f"vaug{par}")
            for g in range(2):
                psv = psT.tile([128, 512], BF16, name="pstb", tag="pstr")
                for k in range(4):
                    kvt = 4 * g + k
                    nc.tensor.matmul(psv[:, k * 128:(k + 1) * 128],
                                     hT[6][:, kvt * 128:(kvt + 1) * 128],
                                     ident_b[:], is_transpose=True,
                                     start=True, stop=True)
                src = psv[:].rearrange("p (k f) -> p k f", k=4)
                nc.vector.tensor_copy(vaug[:, 4 * g:4 * g + 4, 0:64],
                                      src[:, :, 0:64])
                nc.scalar.copy(vaug[:, 4 * g:4 * g + 4, 65:129],
                               src[:, :, 64:128])
            nc.vector.memset(vaug[:, :, 64:65], 1.0)
            nc.vector.memset(vaug[:, :, 129:130], 1.0)
            return vaug

        def emit_attn_qc(hT, vaug, xaT, qc):
            psavs = [psV.tile([65, 512], F32, name=f"av{hh}",
                              tag=f"av{hh}") for hh in range(2)]
            pipe = []          # [(kvt, [ptk_h0, ptk_h1])]
            for kvt in range(8):
                cur = []
                for hh in range(HEADS_PER_CORE):
                    lo, hi = hh * 64, (hh + 1) * 64
                    ps = psA.tile([128, 512], F32, name="mm", tag="mm")
                    nc.tensor.matmul(
                        ps[:], hT[5][lo:hi, kvt * 128:(kvt + 1) * 128],
                        hT[4][lo:hi, qc * 512:(qc + 1) * 512],
                        start=True, stop=True)
                    ptk = ptpool.tile([128, 512], BF16, name=f"PT{hh}",
                                      tag=f"PT{hh}")
                    nc.scalar.activation(ptk[:], ps[:], AF.Exp,
                                         scale=SCALE)
                    cur.append(ptk)
                pipe.append((kvt, cur))
                if len(pipe) > 3:
                    okvt, tiles = pipe.pop(0)
                    for hh in range(HEADS_PER_CORE):
                        nc.tensor.matmul(
                            psavs[hh][:],
                            vaug[:, okvt, hh * 65:hh * 65 + 65],
                            tiles[hh][:],
                            start=(okvt == 0), stop=False)
            for okvt, tiles in pipe:
                for hh in range(HEADS_PER_CORE):
                    nc.tensor.matmul(
                        psavs[hh][:],
                        vaug[:, okvt, hh * 65:hh * 65 + 65],
                        tiles[hh][:],
                        start=(okvt == 0), stop=(okvt == 7))
            for hh in range(HEADS_PER_CORE):
                lo, hi = hh * 64, (hh + 1) * 64
                psav = psavs[hh]
                # rows 0:64 = AV, row 64 = softmax denominator
                rec = small.tile([1, 512], F32R, name="rec", tag="rec")
                nc.vector.reciprocal(rec[:], psav[64:65, :])
                psb = psA.tile([64, 512], F32, name="mm", tag="mm")
                nc.tensor.matmul(psb[:], ones_r[:], rec[:],
                                 start=True, stop=True)
                bc = small.tile([64, 512], F32, name="bc_sb", tag="bc_sb")
                nc.vector.tensor_copy(bc[:], psb[:])
                nc.vector.tensor_tensor(
                    xaT[lo:hi, qc * 512:(qc + 1) * 512],
                    psav[0:64, :], bc[:], op=ALU.mult)

        def emit_outproj_qc(b, hT, xaT, qc):
            for ot in range(8):
                ps = psA.tile([128, 512], F32, name="mm", tag="mm")
                for kk in range(KK):
                    rhs = hT[kk] if kk < 4 else xaT
                    nc.tensor.matmul(
                        ps[:], wfoldT[:, kk, ot * 128:(ot + 1) * 128],
                        rhs[:, qc * 512:(qc + 1) * 512],
                        start=(kk == 0), stop=(kk == KK - 1))
                osb = work.tile([128, 512], BF16, name="osb", tag="osbh")
                if ot % 2:
                    nc.scalar.copy(osb[:], ps[:])
                else:
                    nc.vector.tensor_copy(osb[:], ps[:])
                nc.sync.dma_start(
                    o_d.ap()[ot * 128:(ot + 1) * 128,
                             b * N + qc * 512: b * N + (qc + 1) * 512],
                    osb[:])

        # ---------------- emission order --------------------------------
        def alloc_xnT(par):
            return wpool.tile([128, 8, N], BF16, name="xnT",
                              tag=f"xnTp{par}")

        def alloc_hT(par):
            return [wpool.tile([128, N], BF16, name=f"hT{f}",
                               tag=f"hT{f}p{par}") for f in range(F_TILES)]

        def alloc_xaT(par):
            return wpool.tile([128, N], BF16, name="xaT", tag=f"xaTp{par}")

        # BASS_REPEAT>1 re-emits the body k times for wall-delta timing
        for _rep in range(int(os.environ.get("BASS_REPEAT", "1"))):
            xnT = [alloc_xnT(0), alloc_xnT(1)]
            hT = [alloc_hT(0), alloc_hT(1)]
            xaT = [alloc_xaT(0), alloc_xaT(1)]
            xns = [None] * 8

            # bootstrap: w1 dequant tiles fill PE while the LN chain streams
            emit_w1_tile(4)
            emit_ln_passA(0, xns)
            emit_w1_tile(5)
            w1_rest = [6, 0, 1, 2, 3]
            for tt in range(8):
                emit_ln_passB_tt(xns, xnT[0], tt)
                if tt % 2 == 1 and w1_rest:
                    emit_w1_tile(w1_rest.pop(0))
            while w1_rest:
                emit_w1_tile(w1_rest.pop(0))
            for b in range(B):
                par = b % 2
                # in_proj, interleaved with the w2 dequant stream on b1
                for i, f in enumerate(F_ORDER):
                    if b == 1:
                        emit_w2_tile(i)
                    emit_inproj_tile(xnT[par], hT[par], f)
                if b == 1:
                    emit_w2_tile(7)
                vaug = emit_vtransp(hT[par], par)
                emit_attn_qc(hT[par], vaug, xaT[par], 0)
                if b > 0:
                    emit_outproj_qc(b - 1, hT[1 - par], xaT[1 - par], 0)
                if b + 1 < B:
                    emit_ln_passA(b + 1, xns)
                emit_attn_qc(hT[par], vaug, xaT[par], 1)
                if b > 0:
                    emit_outproj_qc(b - 1, hT[1 - par], xaT[1 - par], 1)
                if b + 1 < B:
                    emit_ln_passB(xns, xnT[1 - par])
            emit_outproj_qc(3, hT[1], xaT[1], 0)
            emit_outproj_qc(3, hT[1], xaT[1], 1)

    nc.compile()
    _BUILD_CACHE["nc"] = nc
    return nc


def make_in_maps(x, in_codebooks, in_indices, out_codebooks, out_indices):
    """Host-side input marshalling: per-core one-hot index matrices (fp8) and
    block-diagonal codebook tiles (bf16), plus the flattened activations.

    Pure layout/encoding transforms - all arithmetic (dequant sums, GEMMs,
    LN, SDPA) runs on device.
    """
    x4096 = np.ascontiguousarray(np.asarray(x).reshape(TOK, DIM)
                                 .astype(NPBF16))
    in_cb = np.asarray(in_codebooks, np.float32)
    in_idx = np.asarray(in_indices)
    out_cb = np.asarray(out_codebooks, np.float32)
    out_idx = np.asarray(out_indices)
    eye = np.arange(K)

    in_maps = []
    for c in range(NCORES):
        # ---- in_proj: icb (7,128,R,2,8,128), ioh (7,128,R,2,8,64) ----
        rows = np.stack([np.arange(_row_base(c, t), _row_base(c, t) + 128)
                         for t in range(F_TILES)])            # (7,128)
        cl0 = np.array([_row_base(c, t) // 64 for t in range(F_TILES)])

        ivc = in_idx[:, rows, :]                              # (R,7,128,16)
        oh = (ivc[..., None] == eye).astype(NPFP8)            # (R,7,128,16,64)
        # axes: (r, t, (h,m), (d,ci), k) -> (t, ci, k, r, h, d, m)
        oh = oh.reshape(R, F_TILES, 2, 64, 8, 2, K)
        ioh = np.ascontiguousarray(
            oh.transpose(1, 5, 6, 0, 2, 4, 3)                 # t,ci,k,r,h,d,m
            .reshape(F_TILES, 128, R, 2, 8, 64))

        # cb tiles: (r, t, h, d, ci, k, s) from clusters cl0[t]+h
        cl_ids = cl0[:, None] + np.array([0, 1])              # (7,2)
        cbs = in_cb[:, cl_ids]                                # (R,7,2,16,64,64)
        cbs = cbs.reshape(R, F_TILES, 2, 8, 2, K, SUB_IN)
        icb = np.zeros((F_TILES, 128, R, 2, 8, 128), np.float32)
        for ci in range(2):
            # (r,t,h,d,k,s) -> (t,k,r,h,d,s)
            blk = cbs[:, :, :, :, ci].transpose(1, 4, 0, 2, 3, 5)
            icb[:, ci * 64:(ci + 1) * 64, :, :, :,
                ci * 64:(ci + 1) * 64] = blk
        icb = icb.astype(NPBF16)

        # ---- out_proj: ocb (bf16) / ooh (fp8), (8,128,R,2,5,128) ---------
        cols = _chunk_cols(c)
        gcbk = np.array([g // SUB_OUT for g in cols])          # (10,)
        gsub = np.array([(g % SUB_OUT) // 64 for g in cols])   # (10,)

        # per-chunk codebook slices: (R, 16 clusters, 10, K, 64)
        sel = np.empty((R, OUT_CLUSTERS, 10, K, 64), np.float32)
        for i in range(10):
            sel[:, :, i] = out_cb[:, :, gcbk[i], :,
                                  64 * gsub[i]: 64 * gsub[i] + 64]
        # (r, (fh,ot) cluster, (kk,ci), k, s) -> (ot, ci, k, r, fh, kk, s)
        sel = sel.reshape(R, 2, 8, KK, 2, K, 64)
        ocb = np.zeros((8, 128, R, 2, KK, 128), np.float32)
        for ci in range(2):
            # (r,fh,ot,kk,k,s) -> (ot,k,r,fh,kk,s)
            blk = sel[:, :, :, :, ci].transpose(2, 4, 0, 1, 3, 5)
            ocb[:, ci * 64:(ci + 1) * 64, :, :, :,
                ci * 64:(ci + 1) * 64] = blk
        ocb = ocb.astype(NPBF16)

        ov = out_idx[:, :, gcbk]                               # (R,2048,10)
        ooh_raw = (ov[..., None] == eye).astype(NPFP8)         # (R,2048,10,64)
        # rows: (r, (fh,ot,m), (kk,ci), k) -> (ot, ci, k, r, fh, kk, m)
        ooh_raw = ooh_raw.reshape(R, 2, 8, 128, KK, 2, K)
        ooh = np.ascontiguousarray(
            ooh_raw.transpose(2, 5, 6, 0, 1, 4, 3)
            .reshape(8, 128, R, 2, KK, 128))

        cbblob = np.concatenate([
            np.ascontiguousarray(icb).ravel(),
            np.ascontiguousarray(ocb).ravel()])
        ohblob = np.concatenate([ioh.ravel(), ooh.ravel()])
        assert cbblob.shape[0] == CB_TOTAL and cbblob.dtype == NPBF16
        assert ohblob.shape[0] == OH_TOTAL and ohblob.dtype == NPFP8
        in_maps.append({"x4096": x4096, "cbblob": cbblob, "ohblob": ohblob})
    return in_maps


def combine_outputs(x, results):
    o_sum = np.zeros((DIM, TOK), np.float32)
    for rmap in results:
        o_sum += np.asarray(rmap["o_t"]).astype(np.float32)
    out = np.asarray(x).reshape(TOK, DIM).astype(np.float32) + o_sum.T
    return out.reshape(B, N, DIM).astype(np.float32)


def kernel(x, in_codebooks, in_indices, out_codebooks, out_indices):
    nc = _build_nc()
    in_maps = make_in_maps(x, in_codebooks, in_indices,
                           out_codebooks, out_indices)
    res = run_bass_kernel_spmd(nc, in_maps, core_ids=list(range(NCORES)))
    return combine_outputs(x, [res.results[c] for c in range(NCORES)])
